# revision 15
# baseline (speedup 1.0000x reference)
"""CTC loss (nn.CTCLoss, blank=0, reduction='mean', zero_infinity=True) for
T=160, B=64, C=6625, S=25 on 8 TRN2 NeuronCores.

Sharding: data-parallel over batch — 8 of the 64 samples per core.

Algorithm: probability-domain CTC forward DP with periodic rescaling, run
BIDIRECTIONALLY to halve the sequential chain: rows 0-7 of the state tile run
alpha forward from t=0, rows 8-15 run beta backward from t=159 with the
extended-target axis reversed, which makes both recurrences the identical
3-tap stencil

    state_new[s] = state[s-2]*c0[s] + state[s-1]*c1[s] + state[s]*c2[s]

computed as one [16, 51, 3] Vector-engine multiply of the overlapped 3-tap
state view against per-iteration coefficients, then a strided reduce over the
tap axis.  The chains meet at t=80 where loss_b = dot(alpha_80, beta_80)
(invariant in the meeting point).  80 iterations instead of 159.

The host packs the coefficients: for each iteration slot i it gathers the 51
extended-target class scores (plus the skip mask as -1e30 fills and the bwd
time/axis reversal — pure selection/layout, no float math) into
pil[i, row, l, tap]; the device exponentiates chunks of slots on the Scalar
engine, pipelined ahead of the DP loop.  Every 8 steps the running sum is
folded out through a fast approximate reciprocal whose exact value is shipped
to the host, so the rescale cancels exactly in the final log-domain combine.
"""

import ml_dtypes
import numpy as np

import concourse.bacc as bacc
import concourse.bass as bass
import concourse.mybir as mybir
import concourse.tile as tile
from concourse.bass_utils import run_bass_kernel_spmd

T = 160
B = 64
C = 6625
S = 25
L = 2 * S + 1  # 51
NCORES = 8
BLOC = B // NCORES  # 8 samples per core
ROWS = 2 * BLOC  # fwd + bwd chains
NITER = 81  # coefficient slots i=0..80; DP loop runs i=1..80
NORM_EVERY = 16
NNORM = 5  # rescales at i = 15, 31, ..., 79
NEG = -1e30
CHUNKS = [4, 8, 12, 17, 20, 20]  # slot-chunk sizes for the DMA/exp pipeline

F32 = mybir.dt.float32
BF16 = mybir.dt.bfloat16
ALU = mybir.AluOpType
ACTF = mybir.ActivationFunctionType
AXIS = mybir.AxisListType


def build_nc() -> bass.Bass:
    nc = bacc.Bacc("TRN2", target_bir_lowering=False)

    pil_d = nc.dram_tensor("pil", [NITER, ROWS, L * 3], F32, kind="ExternalInput")
    oneh_d = nc.dram_tensor("onehotr", [BLOC, L], BF16, kind="ExternalInput")
    out_d = nc.dram_tensor("outv", [ROWS, NNORM + 2], F32, kind="ExternalOutput")
    st_d = nc.dram_tensor("state", [ROWS, L], BF16, kind="ExternalOutput")

    with tile.TileContext(nc) as tc:
        with (
            tc.tile_pool(name="big", bufs=1) as bigp,
            tc.tile_pool(name="small", bufs=1) as smallp,
            tc.tile_pool(name="tmp", bufs=2) as tmpp,
        ):
            pilin = bigp.tile([ROWS, NITER, L * 3], F32, tag="pilin")
            PPQ = bigp.tile([ROWS, NITER, L * 3], BF16, tag="PPQ")

            X = smallp.tile([ROWS, L + 2], BF16, tag="X")
            Y = smallp.tile([ROWS, L + 2], BF16, tag="Y")
            rall = smallp.tile([ROWS, NNORM + 2], F32, tag="rall")
            ssum = smallp.tile([ROWS, NNORM], F32, tag="ssum")
            dummy = smallp.tile([1, 1], F32, tag="dummy")

            # Hoist the Exp act-table load to t~0: a dependency-free dummy
            # activation makes the scheduler place ACT_TABLE_LOAD before any
            # data arrives.
            nc.vector.memset(dummy[:, :], 0.0)
            nc.scalar.activation(dummy[:, :], dummy[:, :], ACTF.Exp)

            nc.vector.memset(X[:, :], 0.0)
            nc.vector.memset(Y[:, :], 0.0)
            nc.vector.memset(rall[:, :], 0.0)

            # bwd init: G_0[sig] = reversed end-state indicator (ACT queue so
            # it overlaps the chunk-0 coefficient DMA on the SP queue).
            nc.scalar.dma_start(out=X[BLOC:ROWS, 2 : L + 2], in_=oneh_d[:, :])

            # Coefficient pipeline: DMA slot-chunk -> exp on ACT.
            s0 = 0
            for ns in CHUNKS:
                in_ap = bass.AP(
                    pil_d,
                    s0 * ROWS * L * 3,
                    [[L * 3, ROWS], [ROWS * L * 3, ns], [1, L * 3]],
                )
                nc.sync.dma_start(out=pilin[:, s0 : s0 + ns, :], in_=in_ap)
                nc.scalar.activation(
                    PPQ[:, s0 : s0 + ns, :], pilin[:, s0 : s0 + ns, :], ACTF.Exp
                )
                s0 += ns

            # fwd init: alpha_0[s=0,1] = p(t=0, l=0,1) = exp'd slot-0 tap 1.
            a0 = PPQ[0:BLOC, 0, :]
            a0v = bass.AP(a0.tensor, a0.offset + 1, [a0.ap[0], [3, 2]])
            nc.vector.tensor_copy(X[0:BLOC, 2:4], a0v)

            cur, nxt = X, Y
            apply_norm = False
            for i in range(1, NITER):
                pp = PPQ[:, i, :]
                ppq_i = bass.AP(pp.tensor, pp.offset, [pp.ap[0], [3, L], [1, 3]])
                xap = cur[:, :]
                xxx = bass.AP(xap.tensor, xap.offset, [xap.ap[0], [1, L], [1, 3]])

                M = tmpp.tile([ROWS, L, 3], BF16, tag="M")
                is_norm = i % NORM_EVERY == NORM_EVERY - 1
                k = i // NORM_EVERY
                if apply_norm or is_norm:
                    nc.vector.scalar_tensor_tensor(
                        out=M[:, :, :],
                        in0=xxx,
                        scalar=rall[:, k - 1 : k] if apply_norm else 1.0,
                        in1=ppq_i,
                        op0=ALU.mult,
                        op1=ALU.mult,
                        accum_out=ssum[:, k : k + 1] if is_norm else None,
                    )
                    apply_norm = False
                else:
                    nc.vector.tensor_tensor(
                        out=M[:, :, :], in0=xxx, in1=ppq_i, op=ALU.mult
                    )
                with nc.allow_low_precision(reason="bf16 DP state; rel tol 2e-2"):
                    nc.vector.tensor_reduce(
                        out=nxt[:, 2 : L + 2], in_=M[:, :, :], axis=AXIS.X, op=ALU.add
                    )
                if is_norm:
                    nc.vector.reciprocal_approx_fast(
                        out=rall[:, k : k + 1], in_=ssum[:, k : k + 1]
                    )
                    apply_norm = True
                cur, nxt = nxt, cur

            # Ship the meeting-point states; the host computes the tiny
            # 51-element dot(alpha_80, reverse(beta_80)) per sample (the
            # pairing crosses partitions, which DVE lanes cannot).
            nc.sync.dma_start(out=st_d[:, :], in_=cur[:, 2 : L + 2])
            nc.scalar.dma_start(out=out_d[:, :], in_=rall[:, :])

    nc.finalize()
    return nc


def _strip_same_engine_waits(nc: bass.Bass) -> None:
    """Remove semaphore waits that only re-assert same-engine program order.

    Tile's clock pass makes every engine instruction wait on its own engine's
    tick semaphore (op N waits for op N-1's completion to propagate, ~190ns).
    Engine instruction queues execute in FIFO order, so for engine-executing
    instructions these self-waits are redundant — this is the same-engine
    subset of the (currently disabled) optimize_sems pass.  Waits on other
    engines' sems, DMA-queue sems, and all waits on sequencer-only
    instructions (e.g. the end-of-block drains, which genuinely wait for the
    engine to finish) are preserved.
    """
    self_sems = {
        "EngineType.DVE": "DVE_49",
        "EngineType.Activation": "Activation_49",
    }
    WINDOW = 12  # wait-queue(4) + exec-queue(8) bypass depth
    for fn in nc.m.functions:
        for bb in fn.blocks:
            cross_cooldown = {}  # engine -> ops remaining in bypass window
            for ins in bb.instructions:
                si = ins.sync_info
                if si is None or ins.is_sequencer_only():
                    continue
                eng = str(ins.engine)
                self_sem = self_sems.get(eng)
                if not self_sem:
                    continue
                has_cross = any(w.ant_name != self_sem for w in si.on_wait)
                cd = cross_cooldown.get(eng, WINDOW)  # start-of-block: keep
                if has_cross:
                    cross_cooldown[eng] = WINDOW
                elif cd > 0:
                    cross_cooldown[eng] = cd - 1
                else:
                    kept = [w for w in si.on_wait if w.ant_name != self_sem]
                    if len(kept) != len(si.on_wait):
                        si.on_wait = kept


def host_prep(predictions, targets, target_lengths):
    """Host-side shard + coefficient-layout prep (gather/select/reverse only;
    all float math on the predictions happens on device). Returns per-core
    input maps."""
    predictions = np.asarray(predictions, dtype=np.float32)
    targets = np.asarray(targets)
    target_lengths = np.asarray(target_lengths)

    ext = np.zeros((B, L), dtype=np.int64)
    ext[:, 1::2] = targets
    skip = np.zeros((B, L), dtype=bool)
    skip[:, 3::2] = targets[:, 1:] != targets[:, :-1]

    # sc[t, b, l] = predictions[t, b, ext[b, l]]; q = skip-masked scores
    sc = np.take_along_axis(
        predictions, np.broadcast_to(ext[None], (T, B, L)), axis=2
    )
    q = np.where(skip[None], sc, np.float32(NEG))

    pil = np.full((NITER, 2 * B, L, 3), NEG, dtype=np.float32)
    # fwd rows (slot i = time t=i): taps (q[l], p[l], p[l])
    pil[:, :B, :, 0] = q[:NITER]
    pil[:, :B, :, 1] = sc[:NITER]
    pil[:, :B, :, 2] = sc[:NITER]
    # bwd rows (slot i = time t=160-i, i=1..79), state axis reversed:
    # taps at sigma: (q[52-sig], p[51-sig], p[50-sig]); l out of range -> NEG
    sig = np.arange(L)
    for i in range(1, NITER - 1):
        t = T - i
        l0, l1, l2 = 52 - sig, 51 - sig, 50 - sig
        v0 = np.where(l0[None] < L, q[t][:, np.minimum(l0, L - 1)], np.float32(NEG))
        v1 = np.where(l1[None] < L, sc[t][:, np.minimum(l1, L - 1)], np.float32(NEG))
        pil[i, B:, :, 0] = v0
        pil[i, B:, :, 1] = v1
        pil[i, B:, :, 2] = sc[t][:, l2]
    # bwd slot 80: identity step (taps exp -> (0, 0, 1))
    pil[NITER - 1, B:, :, 2] = 0.0

    idx = (2 * target_lengths).astype(np.int64)
    oneh_rev = np.zeros((B, L), dtype=np.float32)
    oneh_rev[np.arange(B), (L - 1) - idx] = 1.0
    oneh_rev[np.arange(B), (L - 1) - (idx - 1)] = 1.0

    in_maps = []
    for c in range(NCORES):
        bsl = slice(c * BLOC, (c + 1) * BLOC)
        bsl2 = slice(B + c * BLOC, B + (c + 1) * BLOC)
        pshard = np.concatenate([pil[:, bsl], pil[:, bsl2]], axis=1)  # [81,16,51,3]
        in_maps.append(
            {
                "pil": np.ascontiguousarray(pshard).reshape(NITER, ROWS, L * 3),
                "onehotr": oneh_rev[bsl].astype(ml_dtypes.bfloat16),
            }
        )
    return in_maps


_NC_CACHE = {}


def kernel(predictions, targets, target_lengths):
    if "nc" not in _NC_CACHE:
        _NC_CACHE["nc"] = build_nc()
    nc = _NC_CACHE["nc"]

    in_maps = host_prep(predictions, targets, target_lengths)
    res = run_bass_kernel_spmd(nc, in_maps, core_ids=list(range(NCORES)))
    return finish(res.results, target_lengths)


def finish(results, target_lengths):
    nlls = []
    with np.errstate(divide="ignore"):
        for r in results:
            outv = r["outv"].reshape(ROWS, NNORM + 2)
            st = r["state"].reshape(ROWS, L).astype(np.float32)
            slog = -np.log(outv[:, :NNORM]).sum(axis=1)  # -sum log rcol per row
            dot = (st[:BLOC] * st[BLOC:, ::-1]).sum(axis=1)
            nlls.append(-(np.log(dot) + slog[:BLOC] + slog[BLOC:]))
    nll = np.concatenate(nlls).astype(np.float32)
    lengths = np.asarray(target_lengths).astype(np.float32)
    per = np.where(nll >= 1e29, np.float32(0.0), nll / lengths)
    return np.array(per.mean(), dtype=np.float32)


# revision 16
# speedup vs baseline: 1.1317x; 1.1317x over previous
"""CTC loss for T=160, B=64, C=6625, S=25 on 8 TRN2 NeuronCores.

v3: bidirectional probability-domain DP (fwd alpha rows 0-7, bwd beta rows
8-15 with reversed state axis) *blocked two time-steps per iteration*: each
DP iteration applies the composition of two elementary 3-tap stencils — a
5-tap stencil

    state_new[s] = sum_j state[s-4+j] * c_j[s],  j = 0..4

so the sequential Vector-engine chain is 40 iterations of (multiply, reduce)
instead of 159.  The pair coefficients c_j are sums of 9 elementary products
exp(u+v) of two raw scores; the host packs (u, v) score pairs (gather /
select / reverse only — no float math), the PE computes u+v (pairwise-sum
matmul against a 0/1 selection matrix into PSUM), the Scalar engine
exponentiates, and 4 Vector adds per chunk assemble the 5 coefficient planes.
Rescaling every 8 pairs ships exact reciprocals to the host, which cancels
them in log domain and computes the final 51-element dot + log.
"""

import ml_dtypes
import numpy as np

import concourse.bacc as bacc
import concourse.bass as bass
import concourse.mybir as mybir
import concourse.tile as tile
from concourse.bass_utils import run_bass_kernel_spmd

T = 160
B = 64
C = 6625
S = 25
L = 2 * S + 1  # 51
NCORES = 8
BLOC = B // NCORES  # 8 samples per core
ROWS = 2 * BLOC  # fwd + bwd chains
NPAIR = 40  # 2-step pairs; fwd t=(2k+1,2k+2), bwd t=(159-2k,158-2k)
NNORM = 5  # rescales at kp = 3, 11, 19, 27, 35
NPL = 10  # 9 product planes + 1 assembly scratch plane
NEG = -1e30
CHUNKS = [2, 6, 8, 12, 12]  # pair-slot chunks for the DMA/PE/exp pipeline
PAD = 4  # left zero-pad of the state tile (5-tap reach)

F32 = mybir.dt.float32
F16 = mybir.dt.float16
BF16 = mybir.dt.bfloat16
ALU = mybir.AluOpType
ACTF = mybir.ActivationFunctionType
AXIS = mybir.AxisListType


def build_nc() -> bass.Bass:
    nc = bacc.Bacc("TRN2", target_bir_lowering=False)

    uv_d = nc.dram_tensor("uv", [NPAIR, 2 * ROWS, 9 * L], F16, kind="ExternalInput")
    w_d = nc.dram_tensor("wsel", [2 * ROWS, ROWS], F16, kind="ExternalInput")
    p0_d = nc.dram_tensor("pil0", [BLOC, 2], F32, kind="ExternalInput")
    oneh_d = nc.dram_tensor("onehotr", [BLOC, L], BF16, kind="ExternalInput")
    out_d = nc.dram_tensor("outv", [ROWS, NNORM + 2], F32, kind="ExternalOutput")
    st_d = nc.dram_tensor("state", [ROWS, L], BF16, kind="ExternalOutput")

    with tile.TileContext(nc) as tc:
        with (
            tc.tile_pool(name="big", bufs=1) as bigp,
            tc.tile_pool(name="small", bufs=1) as smallp,
            tc.tile_pool(name="tmp", bufs=2) as tmpp,
            tc.psum_pool(name="ps", bufs=4) as psp,
        ):
            uvt = bigp.tile([2 * ROWS, NPAIR, 9 * L], F16, tag="uvt")
            prod = bigp.tile([ROWS, NPAIR, NPL, L], BF16, tag="prod")

            W = smallp.tile([2 * ROWS, ROWS], F16, tag="W")
            X = smallp.tile([ROWS, L + PAD], BF16, tag="X")
            Y = smallp.tile([ROWS, L + PAD], BF16, tag="Y")
            rall = smallp.tile([ROWS, NNORM + 2], F32, tag="rall")
            ssum = smallp.tile([ROWS, NNORM], F32, tag="ssum")
            p0t = smallp.tile([BLOC, 2], F32, tag="p0t")
            p0e = smallp.tile([BLOC, 2], F32, tag="p0e")
            dummy = smallp.tile([1, 1], F32, tag="dummy")

            # Hoist the Exp act-table load to t~0 (dependency-free activation).
            nc.vector.memset(dummy[:, :], 0.0)
            nc.scalar.activation(dummy[:, :], dummy[:, :], ACTF.Exp)

            nc.vector.memset(X[:, :], 0.0)
            nc.vector.memset(Y[:, :], 0.0)
            nc.vector.memset(rall[:, :], 0.0)

            # Inits on the ACT queue (parallel to the uv DMAs on SP):
            nc.scalar.dma_start(out=X[BLOC:ROWS, PAD : L + PAD], in_=oneh_d[:, :])
            nc.scalar.dma_start(out=p0t[:, :], in_=p0_d[:, :])
            nc.scalar.activation(p0e[:, :], p0t[:, :], ACTF.Exp)
            nc.vector.tensor_copy(X[0:BLOC, PAD : PAD + 2], p0e[:, :])

            nc.sync.dma_start(out=W[:, :], in_=w_d[:, :])

            # Coefficient pipeline: DMA uv chunk -> PE pair-sums -> exp -> 4
            # assembly adds building the 5 coefficient planes per pair-slot.
            k0 = 0
            for ns in CHUNKS:
                in_ap = bass.AP(
                    uv_d,
                    k0 * (2 * ROWS) * 9 * L,
                    [[9 * L, 2 * ROWS], [2 * ROWS * 9 * L, ns], [1, 9 * L]],
                )
                nc.sync.dma_start(out=uvt[:, k0 : k0 + ns, :], in_=in_ap)
                for kp in range(k0, k0 + ns):
                    ps = psp.tile([ROWS, 9 * L], F32, tag="ps")
                    nc.tensor.matmul(ps[:, :], W[:, :], uvt[:, kp, :])
                    nc.scalar.activation(prod[:, kp, 0:9, :], ps[:, :], ACTF.Exp)
                # assembly (per chunk, slot-strided [16, ns, 51] slices):
                # c1 += P[5]; tmp = P[6]+P[7]; c2 += tmp; c3 += P[8]
                def pl(j):
                    return prod[:, k0 : k0 + ns, j, :]

                nc.vector.tensor_tensor(out=pl(1), in0=pl(1), in1=pl(5), op=ALU.add)
                nc.vector.tensor_tensor(out=pl(9), in0=pl(6), in1=pl(7), op=ALU.add)
                nc.vector.tensor_tensor(out=pl(2), in0=pl(2), in1=pl(9), op=ALU.add)
                nc.vector.tensor_tensor(out=pl(3), in0=pl(3), in1=pl(8), op=ALU.add)
                k0 += ns

            cur, nxt = X, Y
            apply_norm = False
            for kp in range(NPAIR):
                pp = prod[:, kp, 0, :]
                cpl = bass.AP(pp.tensor, pp.offset, [pp.ap[0], [L, 5], [1, L]])
                xap = cur[:, :]
                xxx = bass.AP(xap.tensor, xap.offset, [xap.ap[0], [1, 5], [1, L]])

                M = tmpp.tile([ROWS, 5, L], BF16, tag="M")
                is_norm = kp % 8 == 3
                k = kp // 8
                if apply_norm or is_norm:
                    nc.vector.scalar_tensor_tensor(
                        out=M[:, :, :],
                        in0=xxx,
                        scalar=rall[:, k : k + 1] if apply_norm else 1.0,
                        in1=cpl,
                        op0=ALU.mult,
                        op1=ALU.mult,
                        accum_out=ssum[:, k : k + 1] if is_norm else None,
                    )
                    apply_norm = False
                else:
                    nc.vector.tensor_tensor(
                        out=M[:, :, :], in0=xxx, in1=cpl, op=ALU.mult
                    )
                # reduce over the tap axis: view M as [s, tap] (tap stride L)
                mv = M[:, 0, :]
                mvv = bass.AP(mv.tensor, mv.offset, [mv.ap[0], [1, L], [L, 5]])
                with nc.allow_low_precision(reason="bf16 DP state; rel tol 2e-2"):
                    nc.vector.tensor_reduce(
                        out=nxt[:, PAD : L + PAD], in_=mvv, axis=AXIS.X, op=ALU.add
                    )
                if is_norm:
                    nc.vector.reciprocal_approx_fast(
                        out=rall[:, k : k + 1], in_=ssum[:, k : k + 1]
                    )
                    apply_norm = True
                cur, nxt = nxt, cur

            # Ship meeting-point states + reciprocals; host does the tiny
            # 51-element dot(alpha_80, reverse(beta_80)) and the logs.
            nc.sync.dma_start(out=st_d[:, :], in_=cur[:, PAD : L + PAD])
            nc.scalar.dma_start(out=out_d[:, :], in_=rall[:, :])

    nc.finalize()
    return nc


def host_prep(predictions, targets, target_lengths):
    """Host-side shard + (u, v) score-pair packing: gather / select / reverse
    only; all float arithmetic on predictions happens on device."""
    predictions = np.asarray(predictions, dtype=np.float32)
    targets = np.asarray(targets)
    target_lengths = np.asarray(target_lengths)
    NEGf = np.float32(NEG)

    ext = np.zeros((B, L), dtype=np.int64)
    ext[:, 1::2] = targets
    skip = np.zeros((B, L), dtype=bool)
    skip[:, 3::2] = targets[:, 1:] != targets[:, :-1]
    sc = np.take_along_axis(
        predictions, np.broadcast_to(ext[None], (T, B, L)), axis=2
    ).astype(np.float32)
    lq = np.where(skip[None], sc, NEGf)

    def shift(f, n):
        out = np.full_like(f, NEGf)
        if n == 0:
            return f.copy()
        out[..., n:] = f[..., :-n]
        return out

    def fwd_step(t):
        return lq[t], sc[t], sc[t]

    sig = np.arange(L)
    m0 = 52 - sig < L
    m1 = 51 - sig < L

    def bwd_step(t):
        c0 = np.full((B, L), NEGf, np.float32)
        c1 = np.full((B, L), NEGf, np.float32)
        c0[:, m0] = lq[t][:, (52 - sig)[m0]]
        c1[:, m1] = sc[t][:, (51 - sig)[m1]]
        return c0, c1, sc[t][:, 50 - sig]

    ident = (
        np.full((B, L), NEGf, np.float32),
        np.full((B, L), NEGf, np.float32),
        np.zeros((B, L), np.float32),
    )

    uv = np.full((NPAIR, 2 * B, 2, 9, L), NEGf, dtype=np.float32)
    for kp in range(NPAIR):
        s1, s2 = fwd_step(2 * kp + 1), fwd_step(2 * kp + 2)
        b1 = bwd_step(159 - 2 * kp)
        b2 = bwd_step(158 - 2 * kp) if kp < NPAIR - 1 else ident
        for half, (cs1, cs2) in ((0, (s1, s2)), (1, (b1, b2))):
            c01, c11, c21 = cs1
            c02, c12, c22 = cs2
            U = [c02, c02, c02, c12, c22, c12, c12, c22, c22]
            V = [shift(c01, 2), shift(c11, 2), shift(c21, 2), shift(c21, 1),
                 c21, shift(c01, 1), shift(c11, 1), c01, c11]
            uv[kp, half * B : (half + 1) * B, 0] = np.stack(U, axis=1)
            uv[kp, half * B : (half + 1) * B, 1] = np.stack(V, axis=1)

    idx = (2 * target_lengths).astype(np.int64)
    oneh_rev = np.zeros((B, L), dtype=np.float32)
    oneh_rev[np.arange(B), (L - 1) - idx] = 1.0
    oneh_rev[np.arange(B), (L - 1) - (idx - 1)] = 1.0

    wsel = np.zeros((2 * ROWS, ROWS), dtype=np.float32)
    for r in range(ROWS):
        wsel[2 * r, r] = 1.0
        wsel[2 * r + 1, r] = 1.0

    in_maps = []
    for c in range(NCORES):
        bsl = slice(c * BLOC, (c + 1) * BLOC)
        bsl2 = slice(B + c * BLOC, B + (c + 1) * BLOC)
        shard = np.concatenate([uv[:, bsl], uv[:, bsl2]], axis=1)  # [40,16,2,9,L]
        # interleave u/v rows: row 2r = u of row r, 2r+1 = v
        shard = shard.reshape(NPAIR, 2 * ROWS, 9 * L)
        shard = np.maximum(shard, np.float32(-60000.0)).astype(np.float16)
        in_maps.append(
            {
                "uv": np.ascontiguousarray(shard),
                "wsel": wsel.astype(np.float16),
                "pil0": np.ascontiguousarray(sc[0, bsl, 0:2]),
                "onehotr": oneh_rev[bsl].astype(ml_dtypes.bfloat16),
            }
        )
    return in_maps


_NC_CACHE = {}


def kernel(predictions, targets, target_lengths):
    if "nc" not in _NC_CACHE:
        _NC_CACHE["nc"] = build_nc()
    nc = _NC_CACHE["nc"]

    in_maps = host_prep(predictions, targets, target_lengths)
    res = run_bass_kernel_spmd(nc, in_maps, core_ids=list(range(NCORES)))
    return finish(res.results, target_lengths)


def finish(results, target_lengths):
    nlls = []
    with np.errstate(divide="ignore"):
        for r in results:
            outv = r["outv"].reshape(ROWS, NNORM + 2)
            st = r["state"].reshape(ROWS, L).astype(np.float32)
            slog = -np.log(outv[:, :NNORM]).sum(axis=1)
            dot = (st[:BLOC] * st[BLOC:, ::-1]).sum(axis=1)
            nlls.append(-(np.log(dot) + slog[:BLOC] + slog[BLOC:]))
    nll = np.concatenate(nlls).astype(np.float32)
    lengths = np.asarray(target_lengths).astype(np.float32)
    per = np.where(nll >= 1e29, np.float32(0.0), nll / lengths)
    return np.array(per.mean(), dtype=np.float32)
